# revision 1
# baseline (speedup 1.0000x reference)
"""Trainium2 Bass kernel for nn_DenoisingDiffusion_17025250361520.

Graph denoising-diffusion loss: q_sample noise on adjacency, 2-layer GCN,
N*N pairwise edge MLP, sigmoid, symmetrize, BCE loss vs clean adjacency.

Distribution: row-shard the N=1024 node dim across 8 NeuronCores (128 rows
per core).  Small params replicated.  h1/h2 are all-gathered (transposed
layout).  The N*N edge-MLP runs as: per output row i, a fused
tensor_scalar add+relu produces relu(hj_base^T + hi_i) in [k=128, j=1024]
layout (bf16), then a TensorE matvec with the stationary mlp2 weight
reduces over k.  The p <-> p^T exchange for symmetrization is an AllToAll
of 128x128 blocks + PE transposes.  Each core emits a partial BCE sum;
the host adds the 8 partials.

The q_sample scan of Bernoulli flips collapses to adj XOR parity(masks);
the parity mask is input-independent given t, computed host-side with
jax's threefry on CPU (bit-identical to the reference's draws).  The XOR
itself (input-dependent) runs on device as a tensor_tensor(not_equal).
The diagonal of the parity mask is set to 1 so the XOR also plants the
+I self-loop of the GCN normalization for free.
"""

import numpy as np

N = 1024
NODE_DIM = 11
HIDDEN = 128
TIMESTEPS = 100
BETA_START, BETA_END = 1e-4, 0.02
NCORES = 8
R = N // NCORES  # 128 rows per core

_CACHE = {}


# ----------------------------------------------------------------- host prep
def _parity_mask(t: int) -> np.ndarray:
    """Parity (mod-2 sum) of the q_sample flip masks for steps 0..t.

    Bit-exact with the reference's jax.random draws (threefry is
    platform-deterministic); runs on the CPU backend.
    """
    import jax
    import jax.numpy as jnp

    cpu = jax.devices("cpu")[0]
    with jax.default_device(cpu):
        betas = jnp.linspace(BETA_START, BETA_END, TIMESTEPS, dtype=jnp.float32)
        keys = jax.random.split(jax.random.key(42), t + 1)

        def step(c, kb):
            k, b = kb
            m = jax.random.uniform(k, (N, N)) < b
            return jnp.logical_xor(c, m), None

        par, _ = jax.lax.scan(
            step, jnp.zeros((N, N), bool), (keys, betas[: t + 1])
        )
        par = np.asarray(jax.device_get(par))
    p = np.triu(par, 1).astype(np.float32)
    p = p + p.T
    # diag=1 makes the device-side XOR produce adj_noisy + I directly
    np.fill_diagonal(p, 1.0)
    return p


# ------------------------------------------------------------- device program
def _build_program():
    import concourse.bass as bass
    import concourse.mybir as mybir
    import concourse.tile as tile
    from concourse import bacc
    from concourse.bass import ts

    f32 = mybir.dt.float32
    bf16 = mybir.dt.bfloat16
    AL = mybir.AluOpType
    AF = mybir.ActivationFunctionType
    AX = mybir.AxisListType
    RG = [list(range(NCORES))]

    nc = bacc.Bacc(
        "TRN2", target_bir_lowering=False, debug=False, num_devices=NCORES
    )

    ins = {}

    def din(name, shape):
        ins[name] = nc.dram_tensor(name, shape, f32, kind="ExternalInput").ap()
        return ins[name]

    adj_r = din("adj_r", [R, N])      # this core's rows of the clean adjacency
    p_r = din("p_r", [R, N])          # parity mask rows (diag=1)
    xw1_i = din("xw1", [N, HIDDEN])   # x @ w1 (host, tiny K=11 matmul)
    w2_i = din("w2", [HIDDEN, HIDDEN])
    wi_i = din("wi", [HIDDEN, HIDDEN])
    wj_i = din("wj", [HIDDEN, HIDDEN])
    wv_i = din("wv", [HIDDEN, 1])     # mlp2 weight column
    base_i = din("base", [HIDDEN, 1])  # t_emb @ w_t + mlp1_b
    b2c_i = din("b2c", [HIDDEN, 1])   # mlp2 bias broadcast column
    id_i = din("id128", [128, 128])
    ones_i = din("onescol", [128, 1])
    eps_i = din("epscol", [128, 1])      # 1e-12
    onep_i = din("onepcol", [128, 1])    # 1 + 1e-12
    zero_i = din("zerocol", [128, 1])
    dinvr_i = din("dinvr", [R, 1])       # dinv for this core's rows
    dinvp_i = din("dinvp", [R, NCORES])  # dinv[t*128+p] at [p, t]
    out_ap = nc.dram_tensor("out", [1, 1], f32, kind="ExternalOutput").ap()

    with tile.TileContext(nc) as tc:
        with (
            tc.tile_pool(name="const", bufs=1) as cp,
            tc.tile_pool(name="work", bufs=2) as wp,
            tc.tile_pool(name="hot", bufs=6) as hp,
            tc.tile_pool(name="ps", bufs=1, space="PSUM") as pp,
            tc.tile_pool(name="pl", bufs=1, space="PSUM") as plp,
            tc.tile_pool(name="dram", bufs=1, space="DRAM") as dp,
        ):
            B2C = cp.tile([128, 1], f32)
            nc.sync.dma_start(B2C, b2c_i)

            # ---- constants
            ID = cp.tile([128, 128], f32)
            nc.sync.dma_start(ID, id_i)
            W2f = wp.tile([128, 128], f32)
            nc.scalar.dma_start(W2f, w2_i)
            W2 = cp.tile([128, 128], bf16)
            nc.vector.tensor_copy(W2, W2f)
            WIf = wp.tile([128, 128], f32)
            nc.scalar.dma_start(WIf, wi_i)
            WI = cp.tile([128, 128], bf16)
            nc.vector.tensor_copy(WI, WIf)
            WJf = wp.tile([128, 128], f32)
            nc.scalar.dma_start(WJf, wj_i)
            WJ = cp.tile([128, 128], bf16)
            nc.vector.tensor_copy(WJ, WJf)
            WV = cp.tile([128, 1], f32)
            nc.sync.dma_start(WV, wv_i)
            WVb = cp.tile([128, 1], bf16)
            nc.vector.tensor_copy(WVb, WV)
            BASE = cp.tile([128, 1], f32)
            nc.sync.dma_start(BASE, base_i)
            ONES = cp.tile([128, 1], f32)
            nc.sync.dma_start(ONES, ones_i)
            EPS = cp.tile([128, 1], f32)
            nc.sync.dma_start(EPS, eps_i)
            ONEP = cp.tile([128, 1], f32)
            nc.sync.dma_start(ONEP, onep_i)
            ZERO = cp.tile([128, 1], f32)
            nc.sync.dma_start(ZERO, zero_i)
            dinv = cp.tile([R, 1], f32)
            nc.sync.dma_start(dinv, dinvr_i)
            DINVP = cp.tile([R, NCORES], f32)
            nc.sync.dma_start(DINVP, dinvp_i)

            # ---- stage A: noisy adjacency + normalization scale
            # big input DMAs split across engines/queues for bandwidth
            AR = cp.tile([R, N], f32)
            nc.sync.dma_start(AR[0:64, :], adj_r[0:64, :])
            nc.scalar.dma_start(AR[64:128, :], adj_r[64:128, :])
            PR = wp.tile([R, N], f32)
            nc.sync.dma_start(PR[0:64, :], p_r[0:64, :])
            nc.scalar.dma_start(PR[64:128, :], p_r[64:128, :])
            # adj_noisy + I (binary XOR via not_equal; p_r diag=1 plants I)
            NA = cp.tile([R, N], f32)
            nc.vector.tensor_tensor(NA[:, 0:512], AR[:, 0:512], PR[:, 0:512], AL.not_equal)
            nc.vector.tensor_tensor(NA[:, 512:1024], AR[:, 512:1024], PR[:, 512:1024], AL.not_equal)

            # A^T tiles for lhsT of the two GCN matmuls (0/1 exact in bf16)
            ATS = cp.tile([128, NCORES, 128], bf16)
            for t in range(NCORES):
                ptp = pp.tile([128, 128], f32, tag="tp")
                nc.tensor.transpose(ptp, NA[:, ts(t, 128)], ID)
                if t % 2 == 0:
                    nc.vector.tensor_copy(ATS[:, t, :], ptp)
                else:
                    nc.scalar.copy(ATS[:, t, :], ptp)

            # rhs tiles: dinv_j * (x@w1)[j]
            XW1S = cp.tile([128, NCORES, 128], bf16)
            for t in range(NCORES):
                xt = wp.tile([128, 128], f32, tag="xt")
                nc.sync.dma_start(xt, xw1_i[ts(t, 128), :])
                nc.vector.tensor_scalar(
                    XW1S[:, t, :], xt, DINVP[:, t : t + 1], None, AL.mult
                )

            # ---- GCN layer 1: h1 = relu(dinv_i * sum_t AT_t.T @ XW1S_t)
            ph1 = pp.tile([128, 128], f32, tag="acc", bufs=1)
            for t in range(NCORES):
                nc.tensor.matmul(
                    ph1, ATS[:, t, :], XW1S[:, t, :],
                    start=(t == 0), stop=(t == NCORES - 1),
                )
            h1 = wp.tile([128, 128], f32)
            nc.vector.tensor_scalar(h1, ph1, dinv, 0.0, AL.mult, AL.max)
            h1t_ps = pp.tile([128, 128], f32, tag="tp")
            nc.tensor.transpose(h1t_ps, h1, ID)
            h1t = wp.tile([128, 128], bf16)
            nc.vector.tensor_copy(h1t, h1t_ps)
            h1t_b = dp.tile([128, 128], bf16)
            nc.sync.dma_start(h1t_b, h1t)
            h1t_all = dp.tile([NCORES, 128, 128], bf16)
            nc.gpsimd.collective_compute(
                "AllGather", AL.bypass, replica_groups=RG,
                ins=[h1t_b.opt()], outs=[h1t_all.opt()],
            )
            H1T = cp.tile([128, N], bf16)  # h1^T, all nodes: [h, j]
            for s in range(NCORES):
                eng = (nc.sync, nc.scalar, nc.gpsimd)[s % 3]
                eng.dma_start(H1T[:, ts(s, 128)], h1t_all[s, :, :])

            # ---- GCN layer 2
            H1W2S = cp.tile([128, NCORES, 128], bf16)
            for t in range(NCORES):
                pw = pp.tile([128, 128], f32, tag="tp")
                nc.tensor.matmul(pw, H1T[:, ts(t, 128)], W2, start=True, stop=True)
                nc.vector.tensor_scalar(
                    H1W2S[:, t, :], pw, DINVP[:, t : t + 1], None, AL.mult
                )
            ph2 = pp.tile([128, 128], f32, tag="acc", bufs=1)
            for t in range(NCORES):
                nc.tensor.matmul(
                    ph2, ATS[:, t, :], H1W2S[:, t, :],
                    start=(t == 0), stop=(t == NCORES - 1),
                )
            h2 = wp.tile([128, 128], f32)
            nc.vector.tensor_scalar(h2, ph2, dinv, 0.0, AL.mult, AL.max)
            h2t_ps = pp.tile([128, 128], f32, tag="tp")
            nc.tensor.transpose(h2t_ps, h2, ID)
            h2t = wp.tile([128, 128], bf16)
            nc.vector.tensor_copy(h2t, h2t_ps)
            h2t_b = dp.tile([128, 128], bf16)
            nc.sync.dma_start(h2t_b, h2t)
            h2t_all = dp.tile([NCORES, 128, 128], bf16)
            nc.gpsimd.collective_compute(
                "AllGather", AL.bypass, replica_groups=RG,
                ins=[h2t_b.opt()], outs=[h2t_all.opt()],
            )
            H2T = cp.tile([128, N], bf16)
            for s in range(NCORES):
                eng = (nc.sync, nc.scalar, nc.gpsimd)[s % 3]
                eng.dma_start(H2T[:, ts(s, 128)], h2t_all[s, :, :])

            # ---- edge MLP operands
            # hi^T local: [k, i] = wi.T @ h2_r^T
            phi = pp.tile([128, 128], f32, tag="tp")
            nc.tensor.matmul(phi, WI, h2t, start=True, stop=True)
            HITf = cp.tile([128, 128], f32)
            nc.vector.tensor_copy(HITf, phi)
            # (hj + base)^T all nodes: [k, j] bf16
            HJB = cp.tile([128, N], bf16)
            for hh in range(2):
                pj = plp.tile([128, 512], f32, tag="pj")
                nc.tensor.matmul(
                    pj, WJ, H2T[:, ts(hh, 512)], start=True, stop=True
                )
                nc.vector.tensor_scalar(
                    HJB[:, ts(hh, 512)], pj, BASE, None, AL.add
                )

            # ---- hot loop: logits for 128 local rows x 1024 cols.
            # Stationary operand = fused-relu tile slice (K=128, M=128),
            # moving operand = mlp2 weight column (N=1, FWL on the weight
            # load).  LT[:, jb, i] = logit[i, jb*128 : (jb+1)*128]
            # (block-transposed).  Row halves use separate PSUM tiles so
            # sigmoid + AllToAll of the first half overlap the second
            # half's matmuls.
            LTPa = plp.tile([128, NCORES, R // 2], f32, tag="LTa")
            LTPb = plp.tile([128, NCORES, R // 2], f32, tag="LTb")
            PT0 = cp.tile([128, N], f32)
            PT3 = PT0.rearrange("p (jb i) -> p jb i", i=R)
            a_in1 = dp.tile([NCORES, R, R // 2], f32)
            a_out1 = dp.tile([NCORES, R, R // 2], f32)
            a_in2 = dp.tile([NCORES, R, R // 2], f32)
            a_out2 = dp.tile([NCORES, R, R // 2], f32)
            for half, LTP in ((0, LTPa), (1, LTPb)):
                for ii in range(R // 2):
                    i = half * (R // 2) + ii
                    T = hp.tile([128, N], bf16, tag="T")
                    if i % 10 < 7:
                        nc.vector.tensor_scalar(
                            T, HJB, HITf[:, i : i + 1], 0.0, AL.add, AL.max
                        )
                    else:
                        nc.scalar.activation(
                            T, HJB, AF.Relu, bias=HITf[:, i : i + 1]
                        )
                    for jb in range(NCORES):
                        nc.tensor.matmul(
                            LTP[:, jb, ii : ii + 1], T[:, ts(jb, 128)], WVb,
                            start=True, stop=True,
                        )
                lo, hi = half * (R // 2), (half + 1) * (R // 2)
                nc.scalar.activation(PT3[:, :, lo:hi], LTP, AF.Sigmoid, bias=B2C)
                a_in, a_out = (a_in1, a_out1) if half == 0 else (a_in2, a_out2)
                for s in range(NCORES):
                    eng = (nc.sync, nc.scalar)[s % 2]
                    eng.dma_start(
                        a_in[s, :, :], PT0[:, s * 128 + lo : s * 128 + hi]
                    )
                nc.gpsimd.collective_compute(
                    "AllToAll", AL.bypass, replica_groups=RG,
                    ins=[a_in.opt()], outs=[a_out.opt()],
                )

            # AD = p + p^T (= 2*p_hat): received blocks land row-major via
            # one strided DMA; local blocks un-transpose via PE into one
            # PSUM strip; a single add fuses them.
            TPSA = cp.tile([128, NCORES, 128], f32)
            nc.sync.dma_start(TPSA[:, :, 0 : R // 2], a_out1.rearrange("s m q -> m s q"))
            nc.scalar.dma_start(TPSA[:, :, R // 2 : R], a_out2.rearrange("s m q -> m s q"))
            PSB = plp.tile([128, NCORES, 128], f32, tag="LT")
            for s in range(NCORES):
                nc.tensor.transpose(PSB[:, s, :], PT0[:, ts(s, 128)], ID)
            AD = cp.tile([R, N], f32)
            nc.vector.tensor_add(
                AD, TPSA.rearrange("m s q -> m (s q)"),
                PSB.rearrange("m s q -> m (s q)"),
            )

            # ---- BCE partial: q = adj ? p_hat+eps : 1-p_hat+eps, then
            # sum_j ln(q) via the Ln op's free-dim accumulator.
            Q = wp.tile([R, N], f32, bufs=1)
            nc.vector.tensor_scalar(Q, AD, -0.5, 1.0 + 1e-12, AL.mult, AL.add)
            PHT = wp.tile([R, N], f32, bufs=1)
            nc.vector.tensor_scalar(PHT, AD, 0.5, 1e-12, AL.mult, AL.add)
            ARu8 = wp.tile([R, N], mybir.dt.uint8, bufs=1)
            nc.vector.tensor_copy(ARu8, AR)
            nc.vector.copy_predicated(Q, ARu8, PHT)
            LNQ = wp.tile([R, N], f32, bufs=1)
            rs = wp.tile([R, 1], f32)
            nc.scalar.activation(LNQ, Q, AF.Ln, bias=ZERO, accum_out=rs)
            psc = plp.tile([1, 1], f32, tag="pj")
            nc.tensor.matmul(psc, rs, ONES, start=True, stop=True)
            res = wp.tile([1, 1], f32)
            nc.vector.tensor_copy(res, psc)
            nc.sync.dma_start(out_ap, res)

    nc.compile()
    return nc


def _get_program():
    if "nc" not in _CACHE:
        _CACHE["nc"] = _build_program()
    return _CACHE["nc"]


# ------------------------------------------------------------------ interface
def make_in_maps(inputs):
    """Host prep + sharding: full inputs -> per-core input dicts."""
    x = np.asarray(inputs["x"], np.float32)
    adj = np.asarray(inputs["adj"], np.float32)
    t = int(inputs["t"])
    w1 = np.asarray(inputs["w1"], np.float32)
    mlp1_w = np.asarray(inputs["mlp1_w"], np.float32)
    mlp1_b = np.asarray(inputs["mlp1_b"], np.float32)
    mlp2_w = np.asarray(inputs["mlp2_w"], np.float32)
    mlp2_b = np.asarray(inputs["mlp2_b"], np.float32)
    time_emb = np.asarray(inputs["time_emb"], np.float32)
    w2 = np.asarray(inputs["w2"], np.float32)

    P = _parity_mask(t)
    xw1 = np.ascontiguousarray(x @ w1)
    H = HIDDEN
    wi = np.ascontiguousarray(mlp1_w[:H])
    wj = np.ascontiguousarray(mlp1_w[H : 2 * H])
    w_t = mlp1_w[2 * H :]
    base = (time_emb[t] @ w_t + mlp1_b).astype(np.float32).reshape(H, 1)
    wv = np.ascontiguousarray(mlp2_w.reshape(H, 1))
    b2c = np.full((H, 1), float(mlp2_b[0]), np.float32)
    id128 = np.eye(128, dtype=np.float32)
    onescol = np.ones((128, 1), np.float32)
    epscol = np.full((128, 1), 1e-12, np.float32)
    onepcol = np.full((128, 1), 1.0 + 1e-12, np.float32)
    zerocol = np.zeros((128, 1), np.float32)

    # normalization scale 1/sqrt(deg) of the noisy adjacency + self-loops
    noisy = np.abs(adj - P)  # P has diag=1 -> this includes +I
    dinv = (1.0 / np.sqrt(noisy.sum(axis=1, dtype=np.float32))).astype(np.float32)
    dinvp = np.ascontiguousarray(dinv.reshape(NCORES, R).T)  # [p, t]

    shared = {
        "xw1": xw1, "w2": w2, "wi": wi, "wj": wj, "wv": wv,
        "base": base, "b2c": b2c, "id128": id128, "onescol": onescol,
        "epscol": epscol, "onepcol": onepcol, "zerocol": zerocol,
        "dinvp": dinvp,
    }
    in_maps = []
    for c in range(NCORES):
        rows = slice(c * R, (c + 1) * R)
        in_maps.append(
            {
                "adj_r": np.ascontiguousarray(adj[rows]),
                "p_r": np.ascontiguousarray(P[rows]),
                "dinvr": np.ascontiguousarray(dinv[rows].reshape(R, 1)),
                **shared,
            }
        )
    return in_maps


def run_device(in_maps, **kw):
    from concourse.bass_utils import run_bass_kernel_spmd

    nc = _get_program()
    return run_bass_kernel_spmd(nc, in_maps, list(range(NCORES)), **kw)


def kernel(**inputs) -> np.ndarray:
    in_maps = make_in_maps(inputs)
    res = run_device(in_maps)
    total = sum(float(res.results[c]["out"][0, 0]) for c in range(NCORES))
    loss = -total / float(N * N)
    return np.float32(loss)



# revision 6
# speedup vs baseline: 3584.1962x; 3584.1962x over previous
"""Trainium2 Bass kernel for nn_DenoisingDiffusion_17025250361520.

Graph denoising-diffusion loss: q_sample noise on adjacency, 2-layer GCN,
N*N pairwise edge MLP, sigmoid, symmetrize, BCE loss vs clean adjacency.

Distribution: row-shard the N=1024 node dim across 8 NeuronCores (128 rows
per core).  Small params replicated.  h1/h2 are all-gathered (transposed
layout).  The N*N edge-MLP runs as: per output row i, a fused
tensor_scalar add+relu produces relu(hj_base^T + hi_i) in [k=128, j=1024]
layout (bf16), then a TensorE matvec with the stationary mlp2 weight
reduces over k.  The p <-> p^T exchange for symmetrization is an AllToAll
of 128x128 blocks + PE transposes.  Each core emits a partial BCE sum;
the host adds the 8 partials.

The q_sample scan of Bernoulli flips collapses to adj XOR parity(masks);
the parity mask is input-independent given t, computed host-side with
jax's threefry on CPU (bit-identical to the reference's draws).  The XOR
itself (input-dependent) runs on device as a tensor_tensor(not_equal).
The diagonal of the parity mask is set to 1 so the XOR also plants the
+I self-loop of the GCN normalization for free.

Perf notes vs the first working version:
- a dummy 512B AllGather issues at t~0 to absorb the one-time collective
  rendezvous / launch-skew cost (~15-40us) concurrently with the front.
- a tiny Ln activation at t~0 preloads the scalar engine's Ln table set
  so the tail pays no ACT_TABLE_LOAD.
- adjacency + parity ship as uint8, x@w1 ships pre-scaled by dinv_j in
  bf16 -> ~4x less input DMA; all PE transposes run bf16.
- hot-loop T tiles: 16-deep ring, scalar-produced tiles interleaved
  every ~3 so the in-order PE consumer doesn't hit serial slow runs.
"""

import numpy as np

N = 1024
NODE_DIM = 11
HIDDEN = 128
TIMESTEPS = 100
BETA_START, BETA_END = 1e-4, 0.02
NCORES = 8
R = N // NCORES  # 128 rows per core

_CACHE = {}


# ----------------------------------------------------------------- host prep
def _parity_mask(t: int) -> np.ndarray:
    """Parity (mod-2 sum) of the q_sample flip masks for steps 0..t.

    Bit-exact with the reference's jax.random draws (threefry is
    platform-deterministic); runs on the CPU backend.
    """
    import jax
    import jax.numpy as jnp

    cpu = jax.devices("cpu")[0]
    with jax.default_device(cpu):
        betas = jnp.linspace(BETA_START, BETA_END, TIMESTEPS, dtype=jnp.float32)
        keys = jax.random.split(jax.random.key(42), t + 1)

        def step(c, kb):
            k, b = kb
            m = jax.random.uniform(k, (N, N)) < b
            return jnp.logical_xor(c, m), None

        par, _ = jax.lax.scan(
            step, jnp.zeros((N, N), bool), (keys, betas[: t + 1])
        )
        par = np.asarray(jax.device_get(par))
    p = np.triu(par, 1).astype(np.uint8)
    p = p + p.T
    # diag=1 makes the device-side XOR produce adj_noisy + I directly
    np.fill_diagonal(p, 1)
    return p


# ------------------------------------------------------------- device program
def _build_program():
    import concourse.bass as bass
    import concourse.mybir as mybir
    import concourse.tile as tile
    from concourse import bacc
    from concourse.bass import ts

    f32 = mybir.dt.float32
    bf16 = mybir.dt.bfloat16
    u8 = mybir.dt.uint8
    AL = mybir.AluOpType
    AF = mybir.ActivationFunctionType
    RG = [list(range(NCORES))]

    nc = bacc.Bacc(
        "TRN2", target_bir_lowering=False, debug=False, num_devices=NCORES
    )

    ins = {}

    def din(name, shape, dtype=f32):
        ins[name] = nc.dram_tensor(name, shape, dtype, kind="ExternalInput").ap()
        return ins[name]

    adj_r = din("adj_r", [R, N], u8)   # this core's rows of the clean adjacency
    p_r = din("p_r", [R, N], u8)       # parity mask rows (diag=1)
    xw1d_i = din("xw1d", [N, HIDDEN], bf16)  # dinv_j * (x @ w1)[j]  (host)
    w2_i = din("w2", [HIDDEN, HIDDEN])
    wi_i = din("wi", [HIDDEN, HIDDEN])
    wj_i = din("wj", [HIDDEN, HIDDEN])
    wv_i = din("wv", [HIDDEN, 1])     # mlp2 weight column
    base_i = din("base", [HIDDEN, 1])  # t_emb @ w_t + mlp1_b
    b2c_i = din("b2c", [HIDDEN, 1])   # mlp2 bias broadcast column
    id_i = din("id128", [128, 128], bf16)
    ones_i = din("onescol", [128, 1])
    dinvr_i = din("dinvr", [R, 1])       # dinv for this core's rows
    dinvp_i = din("dinvp", [R, NCORES])  # dinv[t*128+p] at [p, t]
    out_ap = nc.dram_tensor("out", [1, 1], f32, kind="ExternalOutput").ap()

    with tile.TileContext(nc) as tc:
        with (
            tc.tile_pool(name="const", bufs=1) as cp,
            tc.tile_pool(name="work", bufs=2) as wp,
            tc.tile_pool(name="hot", bufs=16) as hp,
            tc.tile_pool(name="ps", bufs=1, space="PSUM") as pp,
            tc.tile_pool(name="pl", bufs=1, space="PSUM") as plp,
            tc.tile_pool(name="dram", bufs=1, space="DRAM") as dp,
        ):
            # ---- warm-up: absorb the first-collective rendezvous cost
            # (launch skew + CC init) concurrently with the front compute.
            warm_s = wp.tile([1, 1], f32)
            nc.vector.memset(warm_s, 0.0)
            warm_in = dp.tile([1, 1], f32)
            nc.gpsimd.dma_start(warm_in, warm_s)
            warm_out = dp.tile([NCORES, 1, 1], f32)
            nc.gpsimd.collective_compute(
                "AllGather", AL.bypass, replica_groups=RG,
                ins=[warm_in.opt()], outs=[warm_out.opt()],
            )

            # ---- big input DMAs first (they gate the critical path)
            AR = cp.tile([R, N], u8)
            nc.sync.dma_start(AR[0:64, :], adj_r[0:64, :])
            nc.scalar.dma_start(AR[64:128, :], adj_r[64:128, :])
            PR = wp.tile([R, N], u8)
            nc.sync.dma_start(PR[0:64, :], p_r[0:64, :])
            nc.scalar.dma_start(PR[64:128, :], p_r[64:128, :])
            # rhs tiles for GCN-1: dinv_j * (x@w1)[j], pre-scaled on host
            XW1S = cp.tile([128, NCORES, 128], bf16)
            nc.gpsimd.dma_start(
                XW1S, xw1d_i.rearrange("(t p) h -> p t h", p=128)
            )

            # ---- constants
            B2C = cp.tile([128, 1], f32)
            nc.sync.dma_start(B2C, b2c_i)
            ID = cp.tile([128, 128], bf16)
            nc.sync.dma_start(ID, id_i)
            W2f = wp.tile([128, 128], f32)
            nc.scalar.dma_start(W2f, w2_i)
            W2 = cp.tile([128, 128], bf16)
            nc.vector.tensor_copy(W2, W2f)
            WIf = wp.tile([128, 128], f32)
            nc.scalar.dma_start(WIf, wi_i)
            WI = cp.tile([128, 128], bf16)
            nc.vector.tensor_copy(WI, WIf)
            WJf = wp.tile([128, 128], f32)
            nc.scalar.dma_start(WJf, wj_i)
            WJ = cp.tile([128, 128], bf16)
            nc.vector.tensor_copy(WJ, WJf)
            WV = cp.tile([128, 1], f32)
            nc.sync.dma_start(WV, wv_i)
            WVb = cp.tile([128, 1], bf16)
            nc.vector.tensor_copy(WVb, WV)
            BASE = cp.tile([128, 1], f32)
            nc.sync.dma_start(BASE, base_i)
            ONES = cp.tile([128, 1], f32)
            nc.sync.dma_start(ONES, ones_i)
            dinv = cp.tile([R, 1], f32)
            nc.sync.dma_start(dinv, dinvr_i)
            DINVP = cp.tile([R, NCORES], f32)
            nc.sync.dma_start(DINVP, dinvp_i)

            # preload the Ln activation table set (tail then skips the
            # ~1.3us ACT_TABLE_LOAD on the critical path)
            LnW = wp.tile([1, 1], f32)
            nc.scalar.activation(LnW, ONES[0:1, :], AF.Ln)

            # ---- stage A: noisy adjacency + normalization scale
            # adj_noisy + I (binary XOR via not_equal; p_r diag=1 plants I)
            NA = cp.tile([R, N], bf16)
            nc.vector.tensor_tensor(NA[:, 0:512], AR[:, 0:512], PR[:, 0:512], AL.not_equal)
            nc.vector.tensor_tensor(NA[:, 512:1024], AR[:, 512:1024], PR[:, 512:1024], AL.not_equal)

            # A^T tiles for lhsT of the two GCN matmuls (0/1 exact in bf16)
            ATS = cp.tile([128, NCORES, 128], bf16)
            for t in range(NCORES):
                ptp = pp.tile([128, 128], bf16, tag="tp")
                nc.tensor.transpose(ptp, NA[:, ts(t, 128)], ID)
                if t % 2 == 0:
                    nc.vector.tensor_copy(ATS[:, t, :], ptp)
                else:
                    nc.scalar.copy(ATS[:, t, :], ptp)

            # ---- GCN layer 1: h1 = relu(dinv_i * sum_t AT_t.T @ XW1S_t)
            ph1 = pp.tile([128, 128], f32, tag="acc", bufs=1)
            for t in range(NCORES):
                nc.tensor.matmul(
                    ph1, ATS[:, t, :], XW1S[:, t, :],
                    start=(t == 0), stop=(t == NCORES - 1),
                )
            h1 = wp.tile([128, 128], bf16)
            nc.vector.tensor_scalar(h1, ph1, dinv, 0.0, AL.mult, AL.max)
            h1t_ps = pp.tile([128, 128], bf16, tag="tp")
            nc.tensor.transpose(h1t_ps, h1, ID)
            h1t = wp.tile([128, 128], bf16)
            nc.vector.tensor_copy(h1t, h1t_ps)
            h1t_b = dp.tile([128, 128], bf16)
            nc.sync.dma_start(h1t_b, h1t)
            h1t_all = dp.tile([NCORES, 128, 128], bf16)
            nc.gpsimd.collective_compute(
                "AllGather", AL.bypass, replica_groups=RG,
                ins=[h1t_b.opt()], outs=[h1t_all.opt()],
            )
            H1T = cp.tile([128, N], bf16)  # h1^T, all nodes: [h, j]
            for s in range(NCORES):
                eng = (nc.sync, nc.scalar, nc.gpsimd)[s % 3]
                eng.dma_start(H1T[:, ts(s, 128)], h1t_all[s, :, :])

            # ---- GCN layer 2
            H1W2S = cp.tile([128, NCORES, 128], bf16)
            for t in range(NCORES):
                pw = pp.tile([128, 128], f32, tag="tp")
                nc.tensor.matmul(pw, H1T[:, ts(t, 128)], W2, start=True, stop=True)
                nc.vector.tensor_scalar(
                    H1W2S[:, t, :], pw, DINVP[:, t : t + 1], None, AL.mult
                )
            ph2 = pp.tile([128, 128], f32, tag="acc", bufs=1)
            for t in range(NCORES):
                nc.tensor.matmul(
                    ph2, ATS[:, t, :], H1W2S[:, t, :],
                    start=(t == 0), stop=(t == NCORES - 1),
                )
            h2 = wp.tile([128, 128], bf16)
            nc.vector.tensor_scalar(h2, ph2, dinv, 0.0, AL.mult, AL.max)
            h2t_ps = pp.tile([128, 128], bf16, tag="tp")
            nc.tensor.transpose(h2t_ps, h2, ID)
            h2t = wp.tile([128, 128], bf16)
            nc.vector.tensor_copy(h2t, h2t_ps)
            h2t_b = dp.tile([128, 128], bf16)
            nc.sync.dma_start(h2t_b, h2t)
            h2t_all = dp.tile([NCORES, 128, 128], bf16)
            nc.gpsimd.collective_compute(
                "AllGather", AL.bypass, replica_groups=RG,
                ins=[h2t_b.opt()], outs=[h2t_all.opt()],
            )
            H2T = cp.tile([128, N], bf16)
            for s in range(NCORES):
                eng = (nc.sync, nc.scalar, nc.gpsimd)[s % 3]
                eng.dma_start(H2T[:, ts(s, 128)], h2t_all[s, :, :])

            # ---- edge MLP operands
            # hi^T local: [k, i] = wi.T @ h2_r^T
            phi = pp.tile([128, 128], f32, tag="tp")
            nc.tensor.matmul(phi, WI, h2t, start=True, stop=True)
            HITf = cp.tile([128, 128], f32)
            nc.vector.tensor_copy(HITf, phi)
            # (hj + base)^T all nodes: [k, j] bf16
            HJB = cp.tile([128, N], bf16)
            for hh in range(2):
                pj = plp.tile([128, 512], f32, tag="pj")
                nc.tensor.matmul(
                    pj, WJ, H2T[:, ts(hh, 512)], start=True, stop=True
                )
                nc.vector.tensor_scalar(
                    HJB[:, ts(hh, 512)], pj, BASE, None, AL.add
                )

            # ---- hot loop: logits for 128 local rows x 1024 cols.
            # Stationary operand = fused-relu tile slice (K=128, M=128),
            # moving operand = mlp2 weight column (N=1, FWL on the weight
            # load).  LT[:, jb, i] = logit[i, jb*128 : (jb+1)*128]
            # (block-transposed).  Row halves use separate PSUM tiles so
            # sigmoid + AllToAll of the first half overlap the second
            # half's matmuls.
            LTPa = plp.tile([128, NCORES, R // 2], f32, tag="LTa")
            LTPb = plp.tile([128, NCORES, R // 2], f32, tag="LTb")
            PT0 = cp.tile([128, N], bf16)
            PT3 = PT0.rearrange("p (jb i) -> p jb i", i=R)
            a_in1 = dp.tile([NCORES, R, R // 2], bf16)
            a_out1 = dp.tile([NCORES, R, R // 2], bf16)
            a_in2 = dp.tile([NCORES, R, R // 2], bf16)
            a_out2 = dp.tile([NCORES, R, R // 2], bf16)
            for half, LTP in ((0, LTPa), (1, LTPb)):
                for ii in range(R // 2):
                    i = half * (R // 2) + ii
                    T = hp.tile([128, N], bf16, tag="T")
                    if i % 10 in (2, 5, 8):
                        nc.scalar.activation(
                            T, HJB, AF.Relu, bias=HITf[:, i : i + 1]
                        )
                    else:
                        nc.vector.tensor_scalar(
                            T, HJB, HITf[:, i : i + 1], 0.0, AL.add, AL.max
                        )
                    for jb in range(NCORES):
                        nc.tensor.matmul(
                            LTP[:, jb, ii : ii + 1], T[:, ts(jb, 128)], WVb,
                            start=True, stop=True,
                        )
                lo, hi = half * (R // 2), (half + 1) * (R // 2)
                nc.scalar.activation(PT3[:, :, lo:hi], LTP, AF.Sigmoid, bias=B2C)
                a_in, a_out = (a_in1, a_out1) if half == 0 else (a_in2, a_out2)
                for s in range(NCORES):
                    eng = (nc.sync, nc.scalar)[s % 2]
                    eng.dma_start(
                        a_in[s, :, :], PT0[:, s * 128 + lo : s * 128 + hi]
                    )
                nc.gpsimd.collective_compute(
                    "AllToAll", AL.bypass, replica_groups=RG,
                    ins=[a_in.opt()], outs=[a_out.opt()],
                )

            # AD = p + p^T (= 2*p_hat): received blocks land row-major via
            # one strided DMA; local blocks un-transpose via PE into one
            # PSUM strip; a single add fuses them.
            TPSA = cp.tile([128, NCORES, 128], bf16)
            nc.sync.dma_start(TPSA[:, :, 0 : R // 2], a_out1.rearrange("s m q -> m s q"))
            nc.scalar.dma_start(TPSA[:, :, R // 2 : R], a_out2.rearrange("s m q -> m s q"))
            PSB = plp.tile([128, NCORES, 128], bf16, tag="LT")
            for s in range(NCORES):
                nc.tensor.transpose(PSB[:, s, :], PT0[:, ts(s, 128)], ID)
            AD = cp.tile([R, N], f32)
            nc.vector.tensor_add(
                AD, TPSA.rearrange("m s q -> m (s q)"),
                PSB.rearrange("m s q -> m (s q)"),
            )

            # ---- BCE partial: q = adj ? p_hat+eps : 1-p_hat+eps, then
            # sum_j ln(q) via the Ln op's free-dim accumulator.
            Q = wp.tile([R, N], f32, bufs=1)
            nc.vector.tensor_scalar(Q, AD, -0.5, 1.0 + 1e-12, AL.mult, AL.add)
            PHT = wp.tile([R, N], f32, bufs=1)
            nc.vector.tensor_scalar(PHT, AD, 0.5, 1e-12, AL.mult, AL.add)
            nc.vector.copy_predicated(Q, AR, PHT)
            LNQ = wp.tile([R, N], f32, bufs=1)
            rs = wp.tile([R, 1], f32)
            nc.scalar.activation(LNQ, Q, AF.Ln, accum_out=rs)
            psc = plp.tile([1, 1], f32, tag="pj")
            nc.tensor.matmul(psc, rs, ONES, start=True, stop=True)
            res = wp.tile([1, 1], f32)
            nc.vector.tensor_copy(res, psc)
            nc.sync.dma_start(out_ap, res)

    nc.compile()
    return nc


def _get_program():
    if "nc" not in _CACHE:
        _CACHE["nc"] = _build_program()
    return _CACHE["nc"]


# ------------------------------------------------------------------ interface
def make_in_maps(inputs):
    """Host prep + sharding: full inputs -> per-core input dicts."""
    import ml_dtypes

    bf16 = ml_dtypes.bfloat16
    x = np.asarray(inputs["x"], np.float32)
    adj = np.asarray(inputs["adj"], np.float32)
    t = int(inputs["t"])
    w1 = np.asarray(inputs["w1"], np.float32)
    mlp1_w = np.asarray(inputs["mlp1_w"], np.float32)
    mlp1_b = np.asarray(inputs["mlp1_b"], np.float32)
    mlp2_w = np.asarray(inputs["mlp2_w"], np.float32)
    mlp2_b = np.asarray(inputs["mlp2_b"], np.float32)
    time_emb = np.asarray(inputs["time_emb"], np.float32)
    w2 = np.asarray(inputs["w2"], np.float32)

    P = _parity_mask(t)  # uint8, diag=1
    adj_u8 = adj.astype(np.uint8)
    xw1 = np.ascontiguousarray(x @ w1)
    H = HIDDEN
    wi = np.ascontiguousarray(mlp1_w[:H])
    wj = np.ascontiguousarray(mlp1_w[H : 2 * H])
    w_t = mlp1_w[2 * H :]
    base = (time_emb[t] @ w_t + mlp1_b).astype(np.float32).reshape(H, 1)
    wv = np.ascontiguousarray(mlp2_w.reshape(H, 1))
    b2c = np.full((H, 1), float(mlp2_b[0]), np.float32)
    id128 = np.eye(128, dtype=bf16)
    onescol = np.ones((128, 1), np.float32)

    # normalization scale 1/sqrt(deg) of the noisy adjacency + self-loops
    noisy = np.abs(adj - P.astype(np.float32))  # P diag=1 -> includes +I
    dinv = (1.0 / np.sqrt(noisy.sum(axis=1, dtype=np.float32))).astype(np.float32)
    dinvp = np.ascontiguousarray(dinv.reshape(NCORES, R).T)  # [p, t]
    xw1d = np.ascontiguousarray((dinv[:, None] * xw1).astype(bf16))

    shared = {
        "xw1d": xw1d, "w2": w2, "wi": wi, "wj": wj, "wv": wv,
        "base": base, "b2c": b2c, "id128": id128, "onescol": onescol,
        "dinvp": dinvp,
    }
    in_maps = []
    for c in range(NCORES):
        rows = slice(c * R, (c + 1) * R)
        in_maps.append(
            {
                "adj_r": np.ascontiguousarray(adj_u8[rows]),
                "p_r": np.ascontiguousarray(P[rows]),
                "dinvr": np.ascontiguousarray(dinv[rows].reshape(R, 1)),
                **shared,
            }
        )
    return in_maps


def run_device(in_maps, **kw):
    from concourse.bass_utils import run_bass_kernel_spmd

    nc = _get_program()
    return run_bass_kernel_spmd(nc, in_maps, list(range(NCORES)), **kw)


def kernel(**inputs) -> np.ndarray:
    in_maps = make_in_maps(inputs)
    res = run_device(in_maps)
    total = sum(float(res.results[c]["out"][0, 0]) for c in range(NCORES))
    loss = -total / float(N * N)
    return np.float32(loss)
